# revision 33
# baseline (speedup 1.0000x reference)
"""Trainium2 Bass kernel for PlainMultiheadAttention + SingLoRA.

Problem: B=2, S=2048, D=768, H=12 heads (hd=64), LoRA rank 16 (SingLoRA:
delta_W = A @ A.T, scaling 4.0, u=1.0).

Sharding: 8 cores = 2 batches x 4 head-groups (3 heads per core).
Each core computes q/k/v projections for its head-group columns (LoRA is
folded into the weights on the host: W_eff = W.T + 4*A@A.T), attention for
its 3 heads, and a partial out-projection ([S, D] against its rows of Wo).
The host sums the 4 partials of each batch (the bias bo is carried by
head-group 0 only).

Device dataflow (per core, all matmuls in float32r = full-rate ~fp32):
  - inputs arrive pre-transposed: xT [768, 2048]
  - qT/kT/vT [192, 2048] head-dim-major projections (scores scale 1/8 is
    folded into Wq/bq on the host)
  - v is re-transposed on the PE into natural [2048, 65*3] layout with a
    ones column per head (the PV matmul then yields softmax sums for free
    in psum row 64)
  - scores are computed transposed: S.T tile [seq_k 128, seq_q 512] =
    kT_tile.T @ qT_chunk; exp on ScalarE (no max subtraction needed:
    |scores/8| < ~6 for these magnitudes, exp is safe in fp32)
  - PV: outT_aug [65, 512] accumulated over 16 seq_k tiles
  - normalize via reciprocal(sums) + gpsimd partition-broadcast + DVE mul
  - out-proj: final [seq 128, 768] = aoutT.T @ Wo_rows (+ bias via the
    ones row of the second aout pack)
"""

import math
import sys

sys.path.insert(0, "/opt/trn_rl_repo")

import numpy as np
from contextlib import ExitStack

import concourse.bass as bass
import concourse.bacc as bacc
import concourse.tile as tile
from concourse import mybir, masks
from concourse.bass_utils import run_bass_kernel_spmd
from concourse.tile_rust import add_dep_helper

F32 = mybir.dt.float32
F32R = mybir.dt.float32r

D = 768
S = 2048
B = 2
H = 12
HD = 64
NCORES = 8
HG = 4          # head-groups = cores per batch
H3 = H // HG    # heads per core = 3
GW = H3 * HD    # head-group width = 192
KT = D // 128   # 6 contraction tiles for projections
ST = S // 128   # 16 seq tiles
QC = S // 512   # 4 query chunks
SCALING = 4.0   # lora_alpha / sqrt(r)
INV_SQRT_HD = 1.0 / math.sqrt(HD)


def r(ap):
    """fp32 -> fp32r view for full-rate PE matmuls."""
    return ap.bitcast(F32R)


def _emit(tc, t):
    nc = tc.nc
    ctx = ExitStack()
    with ctx:
        consts = ctx.enter_context(tc.tile_pool(name="consts", bufs=1))
        xin = ctx.enter_context(tc.tile_pool(name="xin", bufs=14))
        qk = ctx.enter_context(tc.tile_pool(name="qk", bufs=1))
        vtp = ctx.enter_context(tc.tile_pool(name="vtp", bufs=1))
        vnp = ctx.enter_context(tc.tile_pool(name="vnp", bufs=1))
        ep = ctx.enter_context(tc.tile_pool(name="ep", bufs=7))
        aop = ctx.enter_context(tc.tile_pool(name="aop", bufs=1))
        osp = ctx.enter_context(tc.tile_pool(name="osp", bufs=2))
        smp = ctx.enter_context(tc.tile_pool(name="smp", bufs=2))
        actx = ExitStack()
        pp = actx.enter_context(tc.tile_pool(name="pp", bufs=2, space="PSUM"))

        # ---- constants / weights ----
        # 64x64 identity replicated on both partition halves so transposes of
        # sources at base partition 0 or 64 both have a matching-base rhs
        ident_f = consts.tile([128, 64], F32, tag="ident_f")
        masks.make_identity(nc, ident_f[0:64, :])
        masks.make_identity(nc, ident_f[64:128, :])
        ident = consts.tile([128, 64], F32R, tag="ident")
        nc.vector.tensor_copy(ident[0:64, :], ident_f[0:64, :])
        nc.vector.tensor_copy(ident[64:128, :], ident_f[64:128, :])
        ones_col = consts.tile([128, 1], F32, tag="ones_col")
        nc.gpsimd.memset(ones_col[:], 1.0)

        def load_wb(wname, bname):
            w = consts.tile([128, KT, GW], F32R, tag=wname, name=wname)
            nc.sync.dma_start(w[:], t[wname][:])
            b0 = consts.tile([128, 1], F32, tag=bname + "0", name=bname + "0")
            b1 = consts.tile([64, 1], F32, tag=bname + "1", name=bname + "1")
            nc.sync.dma_start(b0[:], t[bname][0:128, :])
            nc.sync.dma_start(b1[:], t[bname][128:GW, :])
            return w, (b0, b1)

        # ---- projections: qT/kT/vT [192, 2048] as packs [128, S] + [64, S] ----
        # pack 0 holds heads 0,1 on partitions 0:128; pack 1 holds head 2 on 0:64
        def project(x_dram, w, biases, out0, out1, dup1=False):
            # x loaded as half-seq tiles [128, 1024] so the seq-half chains
            # release their tiles early -> next input's DMA overlaps compute
            xts = {}
            for half in range(2):
                for kt in range(KT):
                    xt = xin.tile([128, S // 2], F32R, tag="xt",
                                  name=f"xt{kt}_{half}")
                    nc.sync.dma_start(
                        xt[:], x_dram[kt, :, half * 1024 : (half + 1) * 1024]
                    )
                    xts[(kt, half)] = xt
            for half in range(2):
                for pack, (out, m, bias) in enumerate(
                    [(out0, 128, biases[0]), (out1, 64, biases[1])]
                ):
                    c0 = pack * 128
                    for q2 in range(2):
                        qc = half * 2 + q2
                        ps = pp.tile([m, 512], F32, tag="pp")
                        for kt in range(KT):
                            nc.tensor.matmul(
                                ps[:],
                                w[:, kt, c0 : c0 + m],
                                xts[(kt, half)][:, q2 * 512 : (q2 + 1) * 512],
                                start=(kt == 0),
                                stop=(kt == KT - 1),
                            )
                        # psum -> sbuf with per-partition bias add
                        qs = slice(qc * 512, (qc + 1) * 512)
                        nc.vector.tensor_scalar_add(out[0:m, qs], ps[:], bias[:])
                        if pack == 1 and dup1:
                            nc.vector.tensor_scalar_add(
                                out[64:128, qs], ps[:], bias[:]
                            )

        qt0 = qk.tile([128, S], F32R, tag="qt0")
        qt1 = qk.tile([128, S], F32R, tag="qt1")
        kt0 = qk.tile([128, S], F32R, tag="kt0")
        kt1 = qk.tile([128, S], F32R, tag="kt1")
        vt0 = vtp.tile([128, S], F32R, tag="vt0")
        vt1 = vtp.tile([64, S], F32R, tag="vt1")

        wq, bq = load_wb("wq", "bq")
        project(t["xq"], wq, bq, qt0, qt1, dup1=True)
        wk, bk = load_wb("wk", "bk")
        project(t["xk"], wk, bk, kt0, kt1, dup1=True)
        wv, bv = load_wb("wv", "bv")
        project(t["xv"], wv, bv, vt0, vt1)

        wo_a = consts.tile([128, D], F32R, tag="wo_a")
        nc.sync.dma_start(wo_a[:], t["wo_a"][:])
        wo_b = consts.tile([64, D], F32R, tag="wo_b")
        nc.sync.dma_start(wo_b[:], t["wo_b"][:])
        bo_row = consts.tile([1, D], F32, tag="bo_row")
        nc.sync.dma_start(bo_row[:], t["bo"][:])
        bo_bc = consts.tile([128, D], F32, tag="bo_bc")
        nc.gpsimd.partition_broadcast(bo_bc[:], bo_row[:])

        # ---- v natural: [2048, 3*65], col 64 of each head-chunk = ones ----
        vnat = [
            vnp.tile([128, H3 * 65], F32R, tag=f"vnat{i}", name=f"vnat{i}")
            for i in range(ST)
        ]
        for st in range(ST):
            for h in range(H3):
                if h == 0:
                    src, idn = vt0[0:64, st * 128 : (st + 1) * 128], ident[0:64, :]
                elif h == 1:
                    src, idn = vt0[64:128, st * 128 : (st + 1) * 128], ident[64:128, :]
                else:
                    src, idn = vt1[0:64, st * 128 : (st + 1) * 128], ident[0:64, :]
                tp = pp.tile([128, 64], F32R, tag="pp_tp")
                nc.tensor.transpose(tp[:], src, idn)
                nc.vector.tensor_copy(vnat[st][:, h * 65 : h * 65 + 64], tp[:])
                nc.vector.tensor_copy(vnat[st][:, h * 65 + 64 : h * 65 + 65], ones_col[:])

        actx.close()
        bctx = ExitStack()
        stp = bctx.enter_context(tc.tile_pool(name="stp", bufs=3, space="PSUM"))
        pvp = bctx.enter_context(tc.tile_pool(name="pvp", bufs=2, space="PSUM"))

        # ---- attention ----
        # aout packs mirror qt packs: heads 0,1 -> ao0 [128, S]; head 2 -> ao1
        ao0 = aop.tile([128, S], F32R, tag="ao0")
        ao1 = aop.tile([64, S], F32R, tag="ao1")

        def normalize(ax, pv, qsl, tag):
            # drain psum to SBUF first so the pv bank frees immediately (the
            # serial reciprocal otherwise stalls the next qc's PV matmuls and
            # re-throttles the PE clock), then divide rows 0:64 by the sums
            # row via reciprocal + partition-broadcast + multiply
            tot = smp.tile([65, 512], F32, tag="tot", name=f"tot{tag}")
            nc.vector.tensor_copy(tot[:], pv[:])
            recip = smp.tile([1, 512], F32, tag="recip", name=f"rcp{tag}")
            nc.vector.reciprocal(recip[:], tot[64:65, :])
            bc = smp.tile([64, 512], F32, tag="bc", name=f"bc{tag}")
            nc.gpsimd.partition_broadcast(bc[:], recip[:])
            nc.vector.tensor_mul(ax[:, qsl], tot[0:64, :], bc[:])

        # Each iteration fills one [128, 2, 512] score psum tile whose two
        # j-slots are computed by two K=64 matmuls on DIFFERENT partition
        # halves (row groups) -> the PE runs them concurrently. One exp
        # activation covers both slots. PV matmuls are full-row K=128.
        def attend2(kxs, qxs, sts, pvts, vns, qsl, tag):
            # kxs/qxs: per-slot lhsT/rhs APs; sts: per-slot seq_k tile idx;
            # pvts: per-slot pv psum tile; vns: per-slot vnat col offset
            prev = None

            shared = pvts[0] is pvts[1]
            n = len(sts)

            def pv_mms(e, sl, i):
                for j in range(2):
                    st = sl[j]
                    first = i == 0 and (j == 0 or not shared)
                    last = i == n - 1 and (j == 1 or not shared)
                    nc.tensor.matmul(
                        pvts[j][:],
                        vnat[st][:, vns[j] : vns[j] + 65],
                        e[:, j, :],
                        start=first,
                        stop=last,
                    )

            for i in range(len(sts)):
                sp = stp.tile([128, 2, 512], F32, tag="st", name=f"sp{tag}")
                mms = []
                for j in range(2):
                    mms.append(
                        nc.tensor.matmul(
                            sp[:, j, :],
                            kxs[j][:, sts[i][j] * 128 : (sts[i][j] + 1) * 128],
                            qxs[j][:, qsl],
                            start=True,
                            stop=True,
                        )
                    )
                # slot matmuls sit on different PE row groups (base partitions
                # 0/64); serialize them defensively (overlap of M=128 pairs is
                # unproven-safe on HW)
                add_dep_helper(
                    mms[1].ins, mms[0].ins, sync=True,
                    reason="serialize row-group score pair",
                )
                if prev is not None:
                    pv_mms(*prev)
                e = ep.tile([128, 2, 512], F32R, tag="e", name=f"e{tag}")
                nc.scalar.activation(e[:], sp[:], mybir.ActivationFunctionType.Exp)
                prev = (e, sts[i], i)
            pv_mms(*prev)

        for qc in range(QC):
            qsl = slice(qc * 512, (qc + 1) * 512)
            # heads 0 & 1: slot j=0 is h0 (partitions 0:64), j=1 is h1 (64:128)
            pv0 = pvp.tile([65, 512], F32, tag="pv", name=f"pv0_{qc}")
            pv1 = pvp.tile([65, 512], F32, tag="pv", name=f"pv1_{qc}")
            attend2(
                kxs=[kt0[0:64], kt0[64:128]],
                qxs=[qt0[0:64], qt0[64:128]],
                sts=[(st, st) for st in range(ST)],
                pvts=[pv0, pv1],
                vns=[0, 65],
                qsl=qsl,
                tag=f"a{qc}",
            )
            normalize(ao0[0:64], pv0, qsl, f"n0_{qc}")
            normalize(ao0[64:128], pv1, qsl, f"n1_{qc}")

            # head 2: qt1/kt1 hold identical data on both partition halves, so
            # consecutive seq_k tiles (st, st+1) pair up as row groups 0/64
            pv2 = pvp.tile([65, 512], F32, tag="pv", name=f"pv2_{qc}")
            attend2(
                kxs=[kt1[0:64], kt1[64:128]],
                qxs=[qt1[0:64], qt1[64:128]],
                sts=[(2 * s, 2 * s + 1) for s in range(ST // 2)],
                pvts=[pv2, pv2],
                vns=[130, 130],
                qsl=qsl,
                tag=f"b{qc}",
            )
            normalize(ao1[0:64], pv2, qsl, f"n2_{qc}")

            # ---- out-projection for this qc's 4 seq tiles (overlaps the
            # next qc's attention); bias added on the psum->sbuf copy ----
            for st4 in range(4):
                sti = qc * 4 + st4
                ssl = slice(sti * 128, (sti + 1) * 128)
                stage = osp.tile([128, D], F32, tag="ostage", name=f"stg{sti}")
                for half in range(2):
                    hs = slice(half * 384, (half + 1) * 384)
                    ps = stp.tile([128, 384], F32, tag="st", name=f"op{sti}")
                    nc.tensor.matmul(
                        ps[:], ao0[:, ssl], wo_a[:, hs], start=True, stop=False
                    )
                    nc.tensor.matmul(
                        ps[:], ao1[:, ssl], wo_b[:, hs], start=False, stop=True
                    )
                    nc.vector.tensor_add(stage[:, hs], ps[:], bo_bc[:, hs])
                nc.sync.dma_start(t["out"][ssl, :], stage[:])

        bctx.close()


def build_nc():
    nc = bacc.Bacc(
        "TRN2",
        target_bir_lowering=False,
        debug=False,
        enable_asserts=False,
        num_devices=NCORES,
    )
    t = {}
    for name in ("xq", "xk", "xv"):
        t[name] = nc.dram_tensor(name, [KT, 128, S], F32R, kind="ExternalInput").ap()
    for name in ("wq", "wk", "wv"):
        t[name] = nc.dram_tensor(name, [128, KT, GW], F32R, kind="ExternalInput").ap()
    for name in ("bq", "bk", "bv"):
        t[name] = nc.dram_tensor(name, [GW, 1], F32, kind="ExternalInput").ap()
    t["wo_a"] = nc.dram_tensor("wo_a", [128, D], F32R, kind="ExternalInput").ap()
    t["wo_b"] = nc.dram_tensor("wo_b", [64, D], F32R, kind="ExternalInput").ap()
    t["bo"] = nc.dram_tensor("bo", [1, D], F32, kind="ExternalInput").ap()
    t["out"] = nc.dram_tensor("out", [S, D], F32, kind="ExternalOutput").ap()

    with tile.TileContext(nc) as tc:
        _emit(tc, t)
    nc.compile()
    return nc


def make_in_maps(query, key, value, Wq, bq, Aq, Wk, bk, Ak, Wv, bv, Av, Wo, bo):
    """Host-side shard prep. Returns list of 8 per-core input dicts."""
    f32 = np.float32

    def w_eff(W, A, scale=1.0):
        # x @ W.T + (x @ (A@A.T).T) * 4  ==  x @ (W.T + 4*A@A.T)
        We = W.T.astype(f32) + SCALING * (A.astype(f32) @ A.T.astype(f32))
        return (We * scale).astype(f32)

    Weq = w_eff(Wq, Aq, INV_SQRT_HD)  # fold 1/sqrt(hd) into q projection
    Wek = w_eff(Wk, Ak)
    Wev = w_eff(Wv, Av)
    bq_s = (bq.astype(f32) * INV_SQRT_HD).astype(f32)

    in_maps = []
    for core in range(NCORES):
        b, hg = divmod(core, HG)
        cols = slice(GW * hg, GW * (hg + 1))
        m = {}
        for name, x in (("xq", query), ("xk", key), ("xv", value)):
            m[name] = np.ascontiguousarray(x[b].T.astype(f32)).reshape(KT, 128, S)
        for name, We in (("wq", Weq), ("wk", Wek), ("wv", Wev)):
            m[name] = np.ascontiguousarray(
                We[:, cols].reshape(KT, 128, GW).transpose(1, 0, 2)
            )
        m["bq"] = np.ascontiguousarray(bq_s[cols]).reshape(GW, 1)
        m["bk"] = np.ascontiguousarray(bk.astype(f32)[cols]).reshape(GW, 1)
        m["bv"] = np.ascontiguousarray(bv.astype(f32)[cols]).reshape(GW, 1)
        wo_sel = np.ascontiguousarray(Wo.astype(f32)[:, cols].T)  # [192, 768]
        m["wo_a"] = np.ascontiguousarray(wo_sel[0:128])
        m["wo_b"] = np.ascontiguousarray(wo_sel[128:GW])
        m["bo"] = (
            bo.astype(f32).reshape(1, D) if hg == 0 else np.zeros((1, D), f32)
        )
        in_maps.append(m)
    return in_maps


_NC = None


def kernel(**inputs):
    global _NC
    if _NC is None:
        _NC = build_nc()
    in_maps = make_in_maps(**inputs)
    res = run_bass_kernel_spmd(_NC, in_maps, core_ids=list(range(NCORES)))
    outs = [res.results[c]["out"] for c in range(NCORES)]
    full = np.stack(
        [np.sum(outs[b * HG : (b + 1) * HG], axis=0, dtype=np.float32) for b in range(B)]
    )
    return full.astype(np.float32)


# revision 35
# speedup vs baseline: 1.1165x; 1.1165x over previous
"""Trainium2 Bass kernel for PlainMultiheadAttention + SingLoRA.

Problem: B=2, S=2048, D=768, H=12 heads (hd=64), LoRA rank 16 (SingLoRA:
delta_W = A @ A.T, scaling 4.0, u=1.0).

Sharding: 8 cores = 2 batches x 4 head-groups (3 heads per core).
Each core computes q/k/v projections for its head-group columns (LoRA is
folded into the weights on the host: W_eff = W.T + 4*A@A.T), attention for
its 3 heads, and a partial out-projection ([S, D] against its rows of Wo).
The host sums the 4 partials of each batch (the bias bo is carried by
head-group 0 only).

Device dataflow (per core, all matmuls in float32r = full-rate ~fp32):
  - inputs arrive pre-transposed: xT [768, 2048]
  - qT/kT/vT [192, 2048] head-dim-major projections (scores scale 1/8 is
    folded into Wq/bq on the host)
  - v is re-transposed on the PE into natural [2048, 65*3] layout with a
    ones column per head (the PV matmul then yields softmax sums for free
    in psum row 64)
  - scores are computed transposed: S.T tile [seq_k 128, seq_q 512] =
    kT_tile.T @ qT_chunk; exp on ScalarE (no max subtraction needed:
    |scores/8| < ~6 for these magnitudes, exp is safe in fp32)
  - PV: outT_aug [65, 512] accumulated over 16 seq_k tiles
  - normalize via reciprocal(sums) + gpsimd partition-broadcast + DVE mul
  - out-proj: final [seq 128, 768] = aoutT.T @ Wo_rows (+ bias via the
    ones row of the second aout pack)
"""

import math
import sys

sys.path.insert(0, "/opt/trn_rl_repo")

import numpy as np
from contextlib import ExitStack

import concourse.bass as bass
import concourse.bacc as bacc
import concourse.tile as tile
from concourse import mybir, masks
from concourse.bass_utils import run_bass_kernel_spmd
from concourse.tile_rust import add_dep_helper

F32 = mybir.dt.float32
F32R = mybir.dt.float32r

D = 768
S = 2048
B = 2
H = 12
HD = 64
NCORES = 8
HG = 4          # head-groups = cores per batch
H3 = H // HG    # heads per core = 3
GW = H3 * HD    # head-group width = 192
KT = D // 128   # 6 contraction tiles for projections
ST = S // 128   # 16 seq tiles
QC = S // 512   # 4 query chunks
SCALING = 4.0   # lora_alpha / sqrt(r)
INV_SQRT_HD = 1.0 / math.sqrt(HD)


def r(ap):
    """fp32 -> fp32r view for full-rate PE matmuls."""
    return ap.bitcast(F32R)


def _emit(tc, t):
    nc = tc.nc
    ctx = ExitStack()
    with ctx:
        consts = ctx.enter_context(tc.tile_pool(name="consts", bufs=1))
        xin = ctx.enter_context(tc.tile_pool(name="xin", bufs=14))
        qk = ctx.enter_context(tc.tile_pool(name="qk", bufs=1))
        vtp = ctx.enter_context(tc.tile_pool(name="vtp", bufs=1))
        vnp = ctx.enter_context(tc.tile_pool(name="vnp", bufs=1))
        ep = ctx.enter_context(tc.tile_pool(name="ep", bufs=6))
        aop = ctx.enter_context(tc.tile_pool(name="aop", bufs=1))
        osp = ctx.enter_context(tc.tile_pool(name="osp", bufs=2))
        smp = ctx.enter_context(tc.tile_pool(name="smp", bufs=2))
        actx = ExitStack()
        pp = actx.enter_context(tc.tile_pool(name="pp", bufs=2, space="PSUM"))

        # ---- constants / weights ----
        # 64x64 identity replicated on both partition halves so transposes of
        # sources at base partition 0 or 64 both have a matching-base rhs
        ident_f = consts.tile([128, 64], F32, tag="ident_f")
        masks.make_identity(nc, ident_f[0:64, :])
        masks.make_identity(nc, ident_f[64:128, :])
        ident = consts.tile([128, 64], F32R, tag="ident")
        nc.vector.tensor_copy(ident[0:64, :], ident_f[0:64, :])
        nc.vector.tensor_copy(ident[64:128, :], ident_f[64:128, :])
        ones_col = consts.tile([128, 1], F32, tag="ones_col")
        nc.gpsimd.memset(ones_col[:], 1.0)

        def load_wb(wname, bname):
            w = consts.tile([128, KT, GW], F32R, tag=wname, name=wname)
            nc.sync.dma_start(w[:], t[wname][:])
            b0 = consts.tile([128, 1], F32, tag=bname + "0", name=bname + "0")
            b1 = consts.tile([64, 1], F32, tag=bname + "1", name=bname + "1")
            nc.sync.dma_start(b0[:], t[bname][0:128, :])
            nc.sync.dma_start(b1[:], t[bname][128:GW, :])
            return w, (b0, b1)

        # ---- projections: qT/kT/vT [192, 2048] as packs [128, S] + [64, S] ----
        # pack 0 holds heads 0,1 on partitions 0:128; pack 1 holds head 2 on 0:64
        def project(x_dram, w, biases, out0, out1, dup1=False):
            # x loaded as half-seq tiles [128, 1024] so the seq-half chains
            # release their tiles early -> next input's DMA overlaps compute
            xts = {}
            for half in range(2):
                for kt in range(KT):
                    xt = xin.tile([128, S // 2], F32R, tag="xt",
                                  name=f"xt{kt}_{half}")
                    nc.sync.dma_start(
                        xt[:], x_dram[kt, :, half * 1024 : (half + 1) * 1024]
                    )
                    xts[(kt, half)] = xt
            for half in range(2):
                for pack, (out, m, bias) in enumerate(
                    [(out0, 128, biases[0]), (out1, 64, biases[1])]
                ):
                    c0 = pack * 128
                    for q2 in range(2):
                        qc = half * 2 + q2
                        ps = pp.tile([m, 512], F32, tag="pp")
                        for kt in range(KT):
                            nc.tensor.matmul(
                                ps[:],
                                w[:, kt, c0 : c0 + m],
                                xts[(kt, half)][:, q2 * 512 : (q2 + 1) * 512],
                                start=(kt == 0),
                                stop=(kt == KT - 1),
                            )
                        # psum -> sbuf with per-partition bias add
                        qs = slice(qc * 512, (qc + 1) * 512)
                        nc.vector.tensor_scalar_add(out[0:m, qs], ps[:], bias[:])
                        if pack == 1 and dup1:
                            nc.vector.tensor_scalar_add(
                                out[64:128, qs], ps[:], bias[:]
                            )

        qt0 = qk.tile([128, S], F32R, tag="qt0")
        qt1 = qk.tile([128, S], F32R, tag="qt1")
        kt0 = qk.tile([128, S], F32R, tag="kt0")
        kt1 = qk.tile([128, S], F32R, tag="kt1")
        vt0 = vtp.tile([128, S], F32R, tag="vt0")
        vt1 = vtp.tile([64, S], F32R, tag="vt1")

        wk, bk = load_wb("wk", "bk")
        project(t["xk"], wk, bk, kt0, kt1, dup1=True)
        wv, bv = load_wb("wv", "bv")
        project(t["xv"], wv, bv, vt0, vt1)
        wq, bq = load_wb("wq", "bq")

        wo_a = consts.tile([128, D], F32R, tag="wo_a")
        nc.sync.dma_start(wo_a[:], t["wo_a"][:])
        wo_b = consts.tile([64, D], F32R, tag="wo_b")
        nc.sync.dma_start(wo_b[:], t["wo_b"][:])
        bo_row = consts.tile([1, D], F32, tag="bo_row")
        nc.sync.dma_start(bo_row[:], t["bo"][:])
        bo_bc = consts.tile([128, D], F32, tag="bo_bc")
        nc.gpsimd.partition_broadcast(bo_bc[:], bo_row[:])

        # ---- v natural: [2048, 3*65], col 64 of each head-chunk = ones ----
        vnat = [
            vnp.tile([128, H3 * 65], F32R, tag=f"vnat{i}", name=f"vnat{i}")
            for i in range(ST)
        ]
        for st in range(ST):
            for h in range(H3):
                if h == 0:
                    src, idn = vt0[0:64, st * 128 : (st + 1) * 128], ident[0:64, :]
                elif h == 1:
                    src, idn = vt0[64:128, st * 128 : (st + 1) * 128], ident[64:128, :]
                else:
                    src, idn = vt1[0:64, st * 128 : (st + 1) * 128], ident[0:64, :]
                tp = pp.tile([128, 64], F32R, tag="pp_tp")
                nc.tensor.transpose(tp[:], src, idn)
                nc.vector.tensor_copy(vnat[st][:, h * 65 : h * 65 + 64], tp[:])
                nc.vector.tensor_copy(vnat[st][:, h * 65 + 64 : h * 65 + 65], ones_col[:])

        actx.close()
        bctx = ExitStack()
        stp = bctx.enter_context(tc.tile_pool(name="stp", bufs=3, space="PSUM"))
        pvp = bctx.enter_context(tc.tile_pool(name="pvp", bufs=2, space="PSUM"))

        # ---- attention ----
        # aout packs mirror qt packs: heads 0,1 -> ao0 [128, S]; head 2 -> ao1
        ao0 = aop.tile([128, S], F32R, tag="ao0")
        ao1 = aop.tile([64, S], F32R, tag="ao1")

        def normalize(ax, pv, qsl, tag):
            # drain psum to SBUF first so the pv bank frees immediately (the
            # serial reciprocal otherwise stalls the next qc's PV matmuls and
            # re-throttles the PE clock), then divide rows 0:64 by the sums
            # row via reciprocal + partition-broadcast + multiply
            tot = smp.tile([65, 512], F32, tag="tot", name=f"tot{tag}")
            nc.vector.tensor_copy(tot[:], pv[:])
            recip = smp.tile([1, 512], F32, tag="recip", name=f"rcp{tag}")
            nc.vector.reciprocal(recip[:], tot[64:65, :])
            bc = smp.tile([64, 512], F32, tag="bc", name=f"bc{tag}")
            nc.gpsimd.partition_broadcast(bc[:], recip[:])
            nc.vector.tensor_mul(ax[:, qsl], tot[0:64, :], bc[:])

        # Each iteration fills one [128, 2, 512] score psum tile whose two
        # j-slots are computed by two K=64 matmuls on DIFFERENT partition
        # halves (row groups) -> the PE runs them concurrently. One exp
        # activation covers both slots. PV matmuls are full-row K=128.
        def attend2(kxs, qxs, sts, pvts, vns, qsl, tag):
            # kxs/qxs: per-slot lhsT/rhs APs; sts: per-slot seq_k tile idx;
            # pvts: per-slot pv psum tile; vns: per-slot vnat col offset
            prev = None

            shared = pvts[0] is pvts[1]
            n = len(sts)

            def pv_mms(e, sl, i):
                for j in range(2):
                    st = sl[j]
                    first = i == 0 and (j == 0 or not shared)
                    last = i == n - 1 and (j == 1 or not shared)
                    nc.tensor.matmul(
                        pvts[j][:],
                        vnat[st][:, vns[j] : vns[j] + 65],
                        e[:, j, :],
                        start=first,
                        stop=last,
                    )

            for i in range(len(sts)):
                sp = stp.tile([128, 2, 512], F32, tag="st", name=f"sp{tag}")
                mms = []
                for j in range(2):
                    mms.append(
                        nc.tensor.matmul(
                            sp[:, j, :],
                            kxs[j][:, sts[i][j] * 128 : (sts[i][j] + 1) * 128],
                            qxs[j][:, qsl],
                            start=True,
                            stop=True,
                        )
                    )
                # slot matmuls sit on different PE row groups (base partitions
                # 0/64); serialize them defensively (overlap of M=128 pairs is
                # unproven-safe on HW)
                add_dep_helper(
                    mms[1].ins, mms[0].ins, sync=True,
                    reason="serialize row-group score pair",
                )
                if prev is not None:
                    pv_mms(*prev)
                e = ep.tile([128, 2, 512], F32R, tag="e", name=f"e{tag}")
                nc.scalar.activation(e[:], sp[:], mybir.ActivationFunctionType.Exp)
                prev = (e, sts[i], i)
            pv_mms(*prev)

        xq_ts = {}
        for half in range(2):
            for kt in range(KT):
                xqt = xin.tile([128, S // 2], F32R, tag="xt",
                               name=f"xq{kt}_{half}")
                nc.sync.dma_start(
                    xqt[:], t["xq"][kt, :, half * 1024 : (half + 1) * 1024]
                )
                xq_ts[(kt, half)] = xqt

        def project_q(qc):
            half, q2 = qc // 2, qc % 2
            qs = slice(qc * 512, (qc + 1) * 512)
            for pack, (out, m, bias) in enumerate(
                [(qt0, 128, bq[0]), (qt1, 64, bq[1])]
            ):
                c0 = pack * 128
                ps = stp.tile([m, 512], F32, tag="st", name=f"qp{qc}{pack}")
                for kt in range(KT):
                    nc.tensor.matmul(
                        ps[:],
                        wq[:, kt, c0 : c0 + m],
                        xq_ts[(kt, half)][:, q2 * 512 : (q2 + 1) * 512],
                        start=(kt == 0),
                        stop=(kt == KT - 1),
                    )
                nc.vector.tensor_scalar_add(out[0:m, qs], ps[:], bias[:])
                if pack == 1:
                    nc.vector.tensor_scalar_add(out[64:128, qs], ps[:], bias[:])

        for qc in range(QC):
            project_q(qc)
            qsl = slice(qc * 512, (qc + 1) * 512)
            # heads 0 & 1: slot j=0 is h0 (partitions 0:64), j=1 is h1 (64:128)
            pv0 = pvp.tile([65, 512], F32, tag="pv", name=f"pv0_{qc}")
            pv1 = pvp.tile([65, 512], F32, tag="pv", name=f"pv1_{qc}")
            attend2(
                kxs=[kt0[0:64], kt0[64:128]],
                qxs=[qt0[0:64], qt0[64:128]],
                sts=[(st, st) for st in range(ST)],
                pvts=[pv0, pv1],
                vns=[0, 65],
                qsl=qsl,
                tag=f"a{qc}",
            )
            normalize(ao0[0:64], pv0, qsl, f"n0_{qc}")
            normalize(ao0[64:128], pv1, qsl, f"n1_{qc}")

            # head 2: qt1/kt1 hold identical data on both partition halves, so
            # consecutive seq_k tiles (st, st+1) pair up as row groups 0/64
            pv2 = pvp.tile([65, 512], F32, tag="pv", name=f"pv2_{qc}")
            attend2(
                kxs=[kt1[0:64], kt1[64:128]],
                qxs=[qt1[0:64], qt1[64:128]],
                sts=[(2 * s, 2 * s + 1) for s in range(ST // 2)],
                pvts=[pv2, pv2],
                vns=[130, 130],
                qsl=qsl,
                tag=f"b{qc}",
            )
            normalize(ao1[0:64], pv2, qsl, f"n2_{qc}")

            # ---- out-projection for this qc's 4 seq tiles (overlaps the
            # next qc's attention); bias added on the psum->sbuf copy ----
            for st4 in range(4):
                sti = qc * 4 + st4
                ssl = slice(sti * 128, (sti + 1) * 128)
                stage = osp.tile([128, D], F32, tag="ostage", name=f"stg{sti}")
                for half in range(2):
                    hs = slice(half * 384, (half + 1) * 384)
                    ps = stp.tile([128, 384], F32, tag="st", name=f"op{sti}")
                    nc.tensor.matmul(
                        ps[:], ao0[:, ssl], wo_a[:, hs], start=True, stop=False
                    )
                    nc.tensor.matmul(
                        ps[:], ao1[:, ssl], wo_b[:, hs], start=False, stop=True
                    )
                    nc.vector.tensor_add(stage[:, hs], ps[:], bo_bc[:, hs])
                nc.sync.dma_start(t["out"][ssl, :], stage[:])

        bctx.close()


def build_nc():
    nc = bacc.Bacc(
        "TRN2",
        target_bir_lowering=False,
        debug=False,
        enable_asserts=False,
        num_devices=NCORES,
    )
    t = {}
    for name in ("xq", "xk", "xv"):
        t[name] = nc.dram_tensor(name, [KT, 128, S], F32R, kind="ExternalInput").ap()
    for name in ("wq", "wk", "wv"):
        t[name] = nc.dram_tensor(name, [128, KT, GW], F32R, kind="ExternalInput").ap()
    for name in ("bq", "bk", "bv"):
        t[name] = nc.dram_tensor(name, [GW, 1], F32, kind="ExternalInput").ap()
    t["wo_a"] = nc.dram_tensor("wo_a", [128, D], F32R, kind="ExternalInput").ap()
    t["wo_b"] = nc.dram_tensor("wo_b", [64, D], F32R, kind="ExternalInput").ap()
    t["bo"] = nc.dram_tensor("bo", [1, D], F32, kind="ExternalInput").ap()
    t["out"] = nc.dram_tensor("out", [S, D], F32, kind="ExternalOutput").ap()

    with tile.TileContext(nc) as tc:
        _emit(tc, t)
    nc.compile()
    return nc


def make_in_maps(query, key, value, Wq, bq, Aq, Wk, bk, Ak, Wv, bv, Av, Wo, bo):
    """Host-side shard prep. Returns list of 8 per-core input dicts."""
    f32 = np.float32

    def w_eff(W, A, scale=1.0):
        # x @ W.T + (x @ (A@A.T).T) * 4  ==  x @ (W.T + 4*A@A.T)
        We = W.T.astype(f32) + SCALING * (A.astype(f32) @ A.T.astype(f32))
        return (We * scale).astype(f32)

    Weq = w_eff(Wq, Aq, INV_SQRT_HD)  # fold 1/sqrt(hd) into q projection
    Wek = w_eff(Wk, Ak)
    Wev = w_eff(Wv, Av)
    bq_s = (bq.astype(f32) * INV_SQRT_HD).astype(f32)

    in_maps = []
    for core in range(NCORES):
        b, hg = divmod(core, HG)
        cols = slice(GW * hg, GW * (hg + 1))
        m = {}
        for name, x in (("xq", query), ("xk", key), ("xv", value)):
            m[name] = np.ascontiguousarray(x[b].T.astype(f32)).reshape(KT, 128, S)
        for name, We in (("wq", Weq), ("wk", Wek), ("wv", Wev)):
            m[name] = np.ascontiguousarray(
                We[:, cols].reshape(KT, 128, GW).transpose(1, 0, 2)
            )
        m["bq"] = np.ascontiguousarray(bq_s[cols]).reshape(GW, 1)
        m["bk"] = np.ascontiguousarray(bk.astype(f32)[cols]).reshape(GW, 1)
        m["bv"] = np.ascontiguousarray(bv.astype(f32)[cols]).reshape(GW, 1)
        wo_sel = np.ascontiguousarray(Wo.astype(f32)[:, cols].T)  # [192, 768]
        m["wo_a"] = np.ascontiguousarray(wo_sel[0:128])
        m["wo_b"] = np.ascontiguousarray(wo_sel[128:GW])
        m["bo"] = (
            bo.astype(f32).reshape(1, D) if hg == 0 else np.zeros((1, D), f32)
        )
        in_maps.append(m)
    return in_maps


_NC = None


def kernel(**inputs):
    global _NC
    if _NC is None:
        _NC = build_nc()
    in_maps = make_in_maps(**inputs)
    res = run_bass_kernel_spmd(_NC, in_maps, core_ids=list(range(NCORES)))
    outs = [res.results[c]["out"] for c in range(NCORES)]
    full = np.stack(
        [np.sum(outs[b * HG : (b + 1) * HG], axis=0, dtype=np.float32) for b in range(B)]
    )
    return full.astype(np.float32)


# revision 36
# speedup vs baseline: 1.1422x; 1.0230x over previous
"""Trainium2 Bass kernel for PlainMultiheadAttention + SingLoRA.

Problem: B=2, S=2048, D=768, H=12 heads (hd=64), LoRA rank 16 (SingLoRA:
delta_W = A @ A.T, scaling 4.0, u=1.0).

Sharding: 8 cores = 2 batches x 4 head-groups (3 heads per core).
Each core computes q/k/v projections for its head-group columns (LoRA is
folded into the weights on the host: W_eff = W.T + 4*A@A.T), attention for
its 3 heads, and a partial out-projection ([S, D] against its rows of Wo).
The host sums the 4 partials of each batch (the bias bo is carried by
head-group 0 only).

Device dataflow (per core, all matmuls in float32r = full-rate ~fp32):
  - inputs arrive pre-transposed: xT [768, 2048]
  - qT/kT/vT [192, 2048] head-dim-major projections (scores scale 1/8 is
    folded into Wq/bq on the host)
  - v is re-transposed on the PE into natural [2048, 65*3] layout with a
    ones column per head (the PV matmul then yields softmax sums for free
    in psum row 64)
  - scores are computed transposed: S.T tile [seq_k 128, seq_q 512] =
    kT_tile.T @ qT_chunk; exp on ScalarE (no max subtraction needed:
    |scores/8| < ~6 for these magnitudes, exp is safe in fp32)
  - PV: outT_aug [65, 512] accumulated over 16 seq_k tiles
  - normalize via reciprocal(sums) + gpsimd partition-broadcast + DVE mul
  - out-proj: final [seq 128, 768] = aoutT.T @ Wo_rows (+ bias via the
    ones row of the second aout pack)
"""

import math
import sys

sys.path.insert(0, "/opt/trn_rl_repo")

import numpy as np
from contextlib import ExitStack

import concourse.bass as bass
import concourse.bacc as bacc
import concourse.tile as tile
from concourse import mybir, masks
from concourse.bass_utils import run_bass_kernel_spmd
from concourse.tile_rust import add_dep_helper

F32 = mybir.dt.float32
F32R = mybir.dt.float32r

D = 768
S = 2048
B = 2
H = 12
HD = 64
NCORES = 8
HG = 4          # head-groups = cores per batch
H3 = H // HG    # heads per core = 3
GW = H3 * HD    # head-group width = 192
KT = D // 128   # 6 contraction tiles for projections
ST = S // 128   # 16 seq tiles
QC = S // 512   # 4 query chunks
SCALING = 4.0   # lora_alpha / sqrt(r)
INV_SQRT_HD = 1.0 / math.sqrt(HD)


def r(ap):
    """fp32 -> fp32r view for full-rate PE matmuls."""
    return ap.bitcast(F32R)


def _emit(tc, t):
    nc = tc.nc
    ctx = ExitStack()
    with ctx:
        consts = ctx.enter_context(tc.tile_pool(name="consts", bufs=1))
        xin = ctx.enter_context(tc.tile_pool(name="xin", bufs=14))
        qk = ctx.enter_context(tc.tile_pool(name="qk", bufs=1))
        vtp = ctx.enter_context(tc.tile_pool(name="vtp", bufs=1))
        vnp = ctx.enter_context(tc.tile_pool(name="vnp", bufs=1))
        ep = ctx.enter_context(tc.tile_pool(name="ep", bufs=6))
        aop = ctx.enter_context(tc.tile_pool(name="aop", bufs=1))
        osp = ctx.enter_context(tc.tile_pool(name="osp", bufs=2))
        smp = ctx.enter_context(tc.tile_pool(name="smp", bufs=2))
        actx = ExitStack()
        pp = actx.enter_context(tc.tile_pool(name="pp", bufs=2, space="PSUM"))

        # ---- constants / weights ----
        # 64x64 identity replicated on both partition halves so transposes of
        # sources at base partition 0 or 64 both have a matching-base rhs
        ident_f = consts.tile([128, 64], F32, tag="ident_f")
        masks.make_identity(nc, ident_f[0:64, :])
        masks.make_identity(nc, ident_f[64:128, :])
        ident = consts.tile([128, 64], F32R, tag="ident")
        nc.vector.tensor_copy(ident[0:64, :], ident_f[0:64, :])
        nc.vector.tensor_copy(ident[64:128, :], ident_f[64:128, :])
        ones_col = consts.tile([128, 1], F32, tag="ones_col")
        nc.gpsimd.memset(ones_col[:], 1.0)

        def load_wb(wname, bname):
            w = consts.tile([128, KT, GW], F32R, tag=wname, name=wname)
            nc.sync.dma_start(w[:], t[wname][:])
            b0 = consts.tile([128, 1], F32, tag=bname + "0", name=bname + "0")
            b1 = consts.tile([64, 1], F32, tag=bname + "1", name=bname + "1")
            nc.sync.dma_start(b0[:], t[bname][0:128, :])
            nc.sync.dma_start(b1[:], t[bname][128:GW, :])
            return w, (b0, b1)

        # ---- projections: qT/kT/vT [192, 2048] as packs [128, S] + [64, S] ----
        # pack 0 holds heads 0,1 on partitions 0:128; pack 1 holds head 2 on 0:64
        def project(x_dram, w, biases, out0, out1, dup1=False):
            # x loaded as half-seq tiles [128, 1024] so the seq-half chains
            # release their tiles early -> next input's DMA overlaps compute
            xts = {}
            for half in range(2):
                for kt in range(KT):
                    xt = xin.tile([128, S // 2], F32R, tag="xt",
                                  name=f"xt{kt}_{half}")
                    nc.sync.dma_start(
                        xt[:], x_dram[kt, :, half * 1024 : (half + 1) * 1024]
                    )
                    xts[(kt, half)] = xt
            for half in range(2):
                for pack, (out, m, bias) in enumerate(
                    [(out0, 128, biases[0]), (out1, 64, biases[1])]
                ):
                    c0 = pack * 128
                    for q2 in range(2):
                        qc = half * 2 + q2
                        ps = pp.tile([m, 512], F32, tag="pp")
                        for kt in range(KT):
                            nc.tensor.matmul(
                                ps[:],
                                w[:, kt, c0 : c0 + m],
                                xts[(kt, half)][:, q2 * 512 : (q2 + 1) * 512],
                                start=(kt == 0),
                                stop=(kt == KT - 1),
                            )
                        # psum -> sbuf with per-partition bias add
                        qs = slice(qc * 512, (qc + 1) * 512)
                        nc.vector.tensor_scalar_add(out[0:m, qs], ps[:], bias[:])
                        if pack == 1 and dup1:
                            nc.vector.tensor_scalar_add(
                                out[64:128, qs], ps[:], bias[:]
                            )

        qt0 = qk.tile([128, S], F32R, tag="qt0")
        qt1 = qk.tile([128, S], F32R, tag="qt1")
        kt0 = qk.tile([128, S], F32R, tag="kt0")
        kt1 = qk.tile([128, S], F32R, tag="kt1")
        vt0 = vtp.tile([128, S], F32R, tag="vt0")
        vt1 = vtp.tile([64, S], F32R, tag="vt1")

        wk, bk = load_wb("wk", "bk")
        project(t["xk"], wk, bk, kt0, kt1, dup1=True)
        wv, bv = load_wb("wv", "bv")
        project(t["xv"], wv, bv, vt0, vt1)
        wq, bq = load_wb("wq", "bq")

        wo_a = consts.tile([128, D], F32R, tag="wo_a")
        nc.sync.dma_start(wo_a[:], t["wo_a"][:])
        wo_b = consts.tile([64, D], F32R, tag="wo_b")
        nc.sync.dma_start(wo_b[:], t["wo_b"][:])
        bo_row = consts.tile([1, D], F32, tag="bo_row")
        nc.sync.dma_start(bo_row[:], t["bo"][:])
        bo_bc = consts.tile([128, D], F32, tag="bo_bc")
        nc.gpsimd.partition_broadcast(bo_bc[:], bo_row[:])

        # ---- v natural: [2048, 3*65], col 64 of each head-chunk = ones ----
        vnat = [
            vnp.tile([128, H3 * 65], F32R, tag=f"vnat{i}", name=f"vnat{i}")
            for i in range(ST)
        ]
        for st in range(ST):
            for h in range(H3):
                if h == 0:
                    src, idn = vt0[0:64, st * 128 : (st + 1) * 128], ident[0:64, :]
                elif h == 1:
                    src, idn = vt0[64:128, st * 128 : (st + 1) * 128], ident[64:128, :]
                else:
                    src, idn = vt1[0:64, st * 128 : (st + 1) * 128], ident[0:64, :]
                tp = pp.tile([128, 64], F32R, tag="pp_tp")
                nc.tensor.transpose(tp[:], src, idn)
                nc.vector.tensor_copy(vnat[st][:, h * 65 : h * 65 + 64], tp[:])
                nc.vector.tensor_copy(vnat[st][:, h * 65 + 64 : h * 65 + 65], ones_col[:])

        actx.close()
        bctx = ExitStack()
        stp = bctx.enter_context(tc.tile_pool(name="stp", bufs=3, space="PSUM"))
        pvp = bctx.enter_context(tc.tile_pool(name="pvp", bufs=2, space="PSUM"))

        # ---- attention ----
        # aout packs mirror qt packs: heads 0,1 -> ao0 [128, S]; head 2 -> ao1
        ao0 = aop.tile([128, S], F32R, tag="ao0")
        ao1 = aop.tile([64, S], F32R, tag="ao1")

        def normalize(ax, pv, qsl, tag):
            # drain psum to SBUF first so the pv bank frees immediately (the
            # serial reciprocal otherwise stalls the next qc's PV matmuls and
            # re-throttles the PE clock), then divide rows 0:64 by the sums
            # row via reciprocal + partition-broadcast + multiply
            tot = smp.tile([65, 512], F32, tag="tot", name=f"tot{tag}")
            nc.vector.tensor_copy(tot[:], pv[:])
            recip = smp.tile([1, 512], F32, tag="recip", name=f"rcp{tag}")
            nc.vector.reciprocal(recip[:], tot[64:65, :])
            bc = smp.tile([64, 512], F32, tag="bc", name=f"bc{tag}")
            nc.gpsimd.partition_broadcast(bc[:], recip[:])
            nc.vector.tensor_mul(ax[:, qsl], tot[0:64, :], bc[:])

        # Each iteration fills one [128, 2, 512] score psum tile whose two
        # j-slots are computed by two K=64 matmuls on DIFFERENT partition
        # halves (row groups) -> the PE runs them concurrently. One exp
        # activation covers both slots. PV matmuls are full-row K=128.
        def attend2(kxs, qxs, sts, pvts, vns, qsl, tag):
            # kxs/qxs: per-slot lhsT/rhs APs; sts: per-slot seq_k tile idx;
            # pvts: per-slot pv psum tile; vns: per-slot vnat col offset
            prev = None

            shared = pvts[0] is pvts[1]
            n = len(sts)

            def pv_mms(e, sl, i):
                for j in range(2):
                    st = sl[j]
                    first = i == 0 and (j == 0 or not shared)
                    last = i == n - 1 and (j == 1 or not shared)
                    nc.tensor.matmul(
                        pvts[j][:],
                        vnat[st][:, vns[j] : vns[j] + 65],
                        e[:, j, :],
                        start=first,
                        stop=last,
                    )

            for i in range(len(sts)):
                sp = stp.tile([128, 2, 512], F32, tag="st", name=f"sp{tag}")
                mms = []
                for j in range(2):
                    mms.append(
                        nc.tensor.matmul(
                            sp[:, j, :],
                            kxs[j][:, sts[i][j] * 128 : (sts[i][j] + 1) * 128],
                            qxs[j][:, qsl],
                            start=True,
                            stop=True,
                        )
                    )
                # slot matmuls sit on different PE row groups (base partitions
                # 0/64); serialize them defensively (overlap of M=128 pairs is
                # unproven-safe on HW)
                add_dep_helper(
                    mms[1].ins, mms[0].ins, sync=True,
                    reason="serialize row-group score pair",
                )
                if prev is not None:
                    pv_mms(*prev)
                e = ep.tile([128, 2, 512], F32R, tag="e", name=f"e{tag}")
                nc.scalar.activation(e[:], sp[:], mybir.ActivationFunctionType.Exp)
                prev = (e, sts[i], i)
            pv_mms(*prev)

        xq_ts = {}
        for half in range(2):
            for kt in range(KT):
                xqt = xin.tile([128, S // 2], F32R, tag="xt",
                               name=f"xq{kt}_{half}")
                nc.sync.dma_start(
                    xqt[:], t["xq"][kt, :, half * 1024 : (half + 1) * 1024]
                )
                xq_ts[(kt, half)] = xqt

        def project_q(qc):
            half, q2 = qc // 2, qc % 2
            qs = slice(qc * 512, (qc + 1) * 512)
            for pack, (out, m, bias) in enumerate(
                [(qt0, 128, bq[0]), (qt1, 64, bq[1])]
            ):
                c0 = pack * 128
                ps = stp.tile([m, 512], F32, tag="st", name=f"qp{qc}{pack}")
                for kt in range(KT):
                    nc.tensor.matmul(
                        ps[:],
                        wq[:, kt, c0 : c0 + m],
                        xq_ts[(kt, half)][:, q2 * 512 : (q2 + 1) * 512],
                        start=(kt == 0),
                        stop=(kt == KT - 1),
                    )
                nc.vector.tensor_scalar_add(out[0:m, qs], ps[:], bias[:])
                if pack == 1:
                    nc.vector.tensor_scalar_add(out[64:128, qs], ps[:], bias[:])

        project_q(0)
        for qc in range(QC):
            qsl = slice(qc * 512, (qc + 1) * 512)
            # heads 0 & 1: slot j=0 is h0 (partitions 0:64), j=1 is h1 (64:128)
            pv0 = pvp.tile([65, 512], F32, tag="pv", name=f"pv0_{qc}")
            pv1 = pvp.tile([65, 512], F32, tag="pv", name=f"pv1_{qc}")
            attend2(
                kxs=[kt0[0:64], kt0[64:128]],
                qxs=[qt0[0:64], qt0[64:128]],
                sts=[(st, st) for st in range(ST)],
                pvts=[pv0, pv1],
                vns=[0, 65],
                qsl=qsl,
                tag=f"a{qc}",
            )
            normalize(ao0[0:64], pv0, qsl, f"n0_{qc}")
            normalize(ao0[64:128], pv1, qsl, f"n1_{qc}")

            # head 2: qt1/kt1 hold identical data on both partition halves, so
            # consecutive seq_k tiles (st, st+1) pair up as row groups 0/64
            pv2 = pvp.tile([65, 512], F32, tag="pv", name=f"pv2_{qc}")
            attend2(
                kxs=[kt1[0:64], kt1[64:128]],
                qxs=[qt1[0:64], qt1[64:128]],
                sts=[(2 * s, 2 * s + 1) for s in range(ST // 2)],
                pvts=[pv2, pv2],
                vns=[130, 130],
                qsl=qsl,
                tag=f"b{qc}",
            )
            # next qc's q-projection emitted BEFORE this qc's normalize and
            # out-projection: it has no dependency on them, so it fills the
            # PE idle gaps around the qc boundary (serial reciprocals + slot
            # recycling) and keeps the PE clock warm
            if qc + 1 < QC:
                project_q(qc + 1)
            normalize(ao1[0:64], pv2, qsl, f"n2_{qc}")

            # ---- out-projection for this qc's 4 seq tiles (overlaps the
            # next qc's attention); bias added on the psum->sbuf copy ----
            for st4 in range(4):
                sti = qc * 4 + st4
                ssl = slice(sti * 128, (sti + 1) * 128)
                stage = osp.tile([128, D], F32, tag="ostage", name=f"stg{sti}")
                for half in range(2):
                    hs = slice(half * 384, (half + 1) * 384)
                    ps = stp.tile([128, 384], F32, tag="st", name=f"op{sti}")
                    nc.tensor.matmul(
                        ps[:], ao0[:, ssl], wo_a[:, hs], start=True, stop=False
                    )
                    nc.tensor.matmul(
                        ps[:], ao1[:, ssl], wo_b[:, hs], start=False, stop=True
                    )
                    nc.vector.tensor_add(stage[:, hs], ps[:], bo_bc[:, hs])
                nc.sync.dma_start(t["out"][ssl, :], stage[:])

        bctx.close()


def build_nc():
    nc = bacc.Bacc(
        "TRN2",
        target_bir_lowering=False,
        debug=False,
        enable_asserts=False,
        num_devices=NCORES,
    )
    t = {}
    for name in ("xq", "xk", "xv"):
        t[name] = nc.dram_tensor(name, [KT, 128, S], F32R, kind="ExternalInput").ap()
    for name in ("wq", "wk", "wv"):
        t[name] = nc.dram_tensor(name, [128, KT, GW], F32R, kind="ExternalInput").ap()
    for name in ("bq", "bk", "bv"):
        t[name] = nc.dram_tensor(name, [GW, 1], F32, kind="ExternalInput").ap()
    t["wo_a"] = nc.dram_tensor("wo_a", [128, D], F32R, kind="ExternalInput").ap()
    t["wo_b"] = nc.dram_tensor("wo_b", [64, D], F32R, kind="ExternalInput").ap()
    t["bo"] = nc.dram_tensor("bo", [1, D], F32, kind="ExternalInput").ap()
    t["out"] = nc.dram_tensor("out", [S, D], F32, kind="ExternalOutput").ap()

    with tile.TileContext(nc) as tc:
        _emit(tc, t)
    nc.compile()
    return nc


def make_in_maps(query, key, value, Wq, bq, Aq, Wk, bk, Ak, Wv, bv, Av, Wo, bo):
    """Host-side shard prep. Returns list of 8 per-core input dicts."""
    f32 = np.float32

    def w_eff(W, A, scale=1.0):
        # x @ W.T + (x @ (A@A.T).T) * 4  ==  x @ (W.T + 4*A@A.T)
        We = W.T.astype(f32) + SCALING * (A.astype(f32) @ A.T.astype(f32))
        return (We * scale).astype(f32)

    Weq = w_eff(Wq, Aq, INV_SQRT_HD)  # fold 1/sqrt(hd) into q projection
    Wek = w_eff(Wk, Ak)
    Wev = w_eff(Wv, Av)
    bq_s = (bq.astype(f32) * INV_SQRT_HD).astype(f32)

    in_maps = []
    for core in range(NCORES):
        b, hg = divmod(core, HG)
        cols = slice(GW * hg, GW * (hg + 1))
        m = {}
        for name, x in (("xq", query), ("xk", key), ("xv", value)):
            m[name] = np.ascontiguousarray(x[b].T.astype(f32)).reshape(KT, 128, S)
        for name, We in (("wq", Weq), ("wk", Wek), ("wv", Wev)):
            m[name] = np.ascontiguousarray(
                We[:, cols].reshape(KT, 128, GW).transpose(1, 0, 2)
            )
        m["bq"] = np.ascontiguousarray(bq_s[cols]).reshape(GW, 1)
        m["bk"] = np.ascontiguousarray(bk.astype(f32)[cols]).reshape(GW, 1)
        m["bv"] = np.ascontiguousarray(bv.astype(f32)[cols]).reshape(GW, 1)
        wo_sel = np.ascontiguousarray(Wo.astype(f32)[:, cols].T)  # [192, 768]
        m["wo_a"] = np.ascontiguousarray(wo_sel[0:128])
        m["wo_b"] = np.ascontiguousarray(wo_sel[128:GW])
        m["bo"] = (
            bo.astype(f32).reshape(1, D) if hg == 0 else np.zeros((1, D), f32)
        )
        in_maps.append(m)
    return in_maps


_NC = None


def kernel(**inputs):
    global _NC
    if _NC is None:
        _NC = build_nc()
    in_maps = make_in_maps(**inputs)
    res = run_bass_kernel_spmd(_NC, in_maps, core_ids=list(range(NCORES)))
    outs = [res.results[c]["out"] for c in range(NCORES)]
    full = np.stack(
        [np.sum(outs[b * HG : (b + 1) * HG], axis=0, dtype=np.float32) for b in range(B)]
    )
    return full.astype(np.float32)
